# revision 1
# baseline (speedup 1.0000x reference)
"""Trainium2 Bass kernel for nn_Node2Vec (EGNN message passing), 8-core SPMD.

Sharding: nodes split across 8 cores (4096 each); edges assigned to the core
owning their destination (row).  Per layer the updated [h|x] rows (264 f32)
are AllGathered so column-side gathers read a local replica.  Scatter-adds
use selection-matrix matmuls over static 256-node windows; row-side gathers
use the same selection matrices against SBUF-resident node-major h tiles.

DRAM replica layout is partition-major: global row index for node n is
(n//4096)*4096 + (n%128)*32 + (n%4096)//128, so SBUF<->DRAM copies are
contiguous per partition.
"""
import numpy as np

NC = 8
N = 32768
NS = N // NC          # 4096 nodes per core
G = 32                # 128-node groups per core
H = 256
F = 512
VOCAB = 780
BS = 32
ROW = 264             # h(256) | x(3) | pad(5)  (f32 node-major SBUF)
ROWB = 272            # bf16 exchange row: h(256) | x-bits(8) | pad(8)
N_LAYERS = 9
COORDS_RANGE = 30.0

_cache = {}


def _pack_edges(edges, cfg):
    """Assign edges to (core, chunk) slots with static window bases shared
    across cores. Returns bases plus per-core colidx/lr arrays [128, NCH]."""
    row = edges[cfg].astype(np.int64)
    col = edges[1 - cfg].astype(np.int64)

    insts = []
    percore = []
    for c in range(NC):
        m = (row // NS) == c
        r = row[m] - c * NS
        k = col[m]
        order = np.argsort(r, kind="stable")
        r, k = r[order], k[order]
        insts.append(np.bincount(r // 128, minlength=G))
        percore.append((r, k))

    def try_pack(cnt, bases):
        cap = {}
        for kk, g in enumerate(bases):
            cap.setdefault(g, []).append([kk, 128])
        for j in range(G):
            left = int(cnt[j])
            for g in (j - 1, j):
                for slot in cap.get(g, []):
                    t = min(slot[1], left)
                    slot[1] -= t
                    left -= t
                    if left == 0:
                        break
                if left == 0:
                    break
            if left > 0:
                return j
        return -1

    bases = sorted(min(j, 30) for j in range(31))
    for _ in range(200):
        bad = -1
        for cnt in insts:
            rbad = try_pack(cnt, bases)
            if rbad >= 0:
                bad = rbad
                break
        if bad < 0:
            break
        bases.append(min(bad, 30))
        bases.sort()
    else:
        raise RuntimeError("edge packing failed")

    NCH = len(bases)
    colidx = np.zeros((NC, 128, NCH), np.int64)
    lr = np.full((NC, 128, NCH), 300, np.int32)
    for c in range(NC):
        r, k = percore[c]
        grp = r // 128
        cap = {}
        for kk, g in enumerate(bases):
            cap.setdefault(g, []).append([kk, 0])
        for j in range(G):
            idxs = np.nonzero(grp == j)[0]
            pos = 0
            for g in (j - 1, j):
                for slot in cap.get(g, []):
                    while slot[1] < 128 and pos < len(idxs):
                        e = idxs[pos]
                        colidx[c, slot[1], slot[0]] = k[e]
                        lr[c, slot[1], slot[0]] = r[e] - g * 128
                        slot[1] += 1
                        pos += 1
                    if pos == len(idxs):
                        break
                if pos == len(idxs):
                    break
            assert pos == len(idxs), "packing inconsistency"
    return bases, colidx, lr


def _permrow(n):
    """DRAM p-major row index for global node id n."""
    return (n // NS) * NS + (n % 128) * G + (n % NS) // 128


def _prep(inputs):
    f32 = np.float32
    i32 = np.int32
    feature = np.asarray(inputs["feature"], f32).reshape(N, F)
    v = np.asarray(inputs["v"]).astype(i32).reshape(N)
    size = np.asarray(inputs["size"]).astype(i32).reshape(N)
    pos = np.asarray(inputs["pos"], f32).reshape(N, 3)
    edges = np.asarray(inputs["edges"]).astype(np.int64)
    predict_idx = np.asarray(inputs["predict_idx"]).astype(np.int64)
    val = np.asarray(inputs["val"], f32)

    bases0, colidx0, lr0 = _pack_edges(edges, 0)
    bases1, colidx1, lr1 = _pack_edges(edges, 1)
    NCH = max(len(bases0), len(bases1))

    def padcfg(bases, colidx, lr):
        k = NCH - len(bases)
        if k:
            bases = [0] * k + list(bases)
            colidx = np.concatenate([np.zeros((NC, 128, k), np.int64), colidx], 2)
            lr = np.concatenate([np.full((NC, 128, k), 300, i32), lr], 2)
        return bases, colidx, lr

    bases0, colidx0, lr0 = padcfg(bases0, colidx0, lr0)
    bases1, colidx1, lr1 = padcfg(bases1, colidx1, lr1)
    meta = dict(NCH=NCH, bases=(bases0, bases1))

    colperm0 = _permrow(colidx0).astype(i32)
    colperm1 = _permrow(colidx1).astype(i32)

    maps = []
    for c in range(NC):
        sl = slice(c * NS, (c + 1) * NS)
        featT = np.ascontiguousarray(feature[sl].T)           # [512, 4096]
        pos_pm = np.zeros((128, G, 4), f32)                   # p-major
        pos_pm[:, :, :3] = pos[sl].reshape(G, 128, 3).transpose(1, 0, 2)
        vi = np.ascontiguousarray(v[sl].reshape(G, 128).T.astype(i32))
        si = np.ascontiguousarray(size[sl].reshape(G, 128).T.astype(i32))
        lrrow0 = np.ascontiguousarray(lr0[c].T.reshape(1, NCH * 128).astype(f32))
        lrrow1 = np.ascontiguousarray(lr1[c].T.reshape(1, NCH * 128).astype(f32))
        nloc = np.arange(4) * 1024 + predict_idx[4 * c:4 * c + 4]
        ploc = ((nloc % 128) * G + nloc // 128).astype(i32).reshape(4, 1)
        maps.append(dict(
            featT=featT, pos_pm=np.ascontiguousarray(pos_pm.reshape(128, G * 4)),
            v_idx=vi, s_idx=si,
            colidx0=np.ascontiguousarray(colperm0[c]),
            colidx1=np.ascontiguousarray(colperm1[c]),
            lrcol0=np.ascontiguousarray(lr0[c]),
            lrcol1=np.ascontiguousarray(lr1[c]),
            lrrow0=lrrow0, lrrow1=lrrow1,
            pidx=ploc,
            valrow=np.ascontiguousarray(val[4 * c:4 * c + 4].reshape(1, 4)),
        ))

    def wT(x):
        return np.ascontiguousarray(np.asarray(x, f32))

    def bias2(b, nch):
        return np.ascontiguousarray(np.asarray(b, f32).reshape(nch, 128).T)

    shared = dict(
        v_emb=wT(inputs["v_emb"]), size_emb=wT(inputs["size_emb"]),
        fW1=wT(inputs["fW1"]), fW2=wT(inputs["fW2"]),
        pW1=wT(inputs["pW1"]), pW2=wT(inputs["pW2"]), pW3=wT(inputs["pW3"]),
        fb1=bias2(inputs["fb1"], 2), fb2=bias2(inputs["fb2"], 2),
        pb1=bias2(inputs["pb1"], 6), pb2=bias2(inputs["pb2"], 2),
        pb3=bias2(inputs["pb3"], 2),
        We1=wT(inputs["We1"]), We2=wT(inputs["We2"]),
        Wn1=wT(inputs["Wn1"]), Wn2=wT(inputs["Wn2"]), Wc1=wT(inputs["Wc1"]),
        be1=np.stack([bias2(np.asarray(inputs["be1"])[l], 2) for l in range(9)]),
        be2=np.stack([bias2(np.asarray(inputs["be2"])[l], 2) for l in range(9)]),
        bn1=np.stack([bias2(np.asarray(inputs["bn1"])[l], 2) for l in range(9)]),
        bn2=np.stack([bias2(np.asarray(inputs["bn2"])[l], 2) for l in range(9)]),
        bc1row=np.ascontiguousarray(np.asarray(inputs["bc1"], f32).reshape(9, 1, H)),
        Wattrow=np.ascontiguousarray(np.asarray(inputs["Watt"], f32).transpose(0, 2, 1)),
        battrow=np.ascontiguousarray(np.broadcast_to(
            np.asarray(inputs["batt"], f32).reshape(9, 1, 1), (9, 128, 1)).copy()),
        Wc2row=np.ascontiguousarray(np.asarray(inputs["Wc2"], f32).transpose(0, 2, 1)),
        oW1=wT(inputs["oW1"]), oW2=wT(inputs["oW2"]),
        ob1=bias2(inputs["ob1"], 2),
        ob2=np.ascontiguousarray(
            np.pad(np.asarray(inputs["ob2"], f32), (0, 128 * 7 - VOCAB)).reshape(7, 128).T),
        ones_row=np.ones((1, 128), f32),
        iota_col=np.arange(128, dtype=f32).reshape(128, 1),
        iota_col128=np.arange(128, 256, dtype=f32).reshape(128, 1),
        iota_row256=np.ascontiguousarray(np.broadcast_to(
            np.arange(256, dtype=i32).reshape(1, 256), (128, 256)).copy()),
    )
    for m in maps:
        m.update(shared)
    return meta, maps


def _build(meta, nl=N_LAYERS, with_head=True, dbg=(), sim1=False):
    import concourse.bacc as bacc
    import concourse.bass as bass
    import concourse.mybir as mybir
    import concourse.tile as tile
    from concourse.masks import make_identity

    dt = mybir.dt
    AF = mybir.ActivationFunctionType
    ALU = mybir.AluOpType
    NCH = meta["NCH"]
    BASES = meta["bases"]

    nc = bacc.Bacc("TRN2", target_bir_lowering=False, debug=False,
                   num_devices=1 if sim1 else NC, enable_asserts=False)

    def din(name, shape, d=dt.float32):
        return nc.dram_tensor(name, list(shape), d, kind="ExternalInput")

    featT = din("featT", [F, NS], dt.float32r)
    pos_pm = din("pos_pm", [128, G * 4])
    v_idx = din("v_idx", [128, G], dt.int32)
    s_idx = din("s_idx", [128, G], dt.int32)
    colidx_t = [din("colidx0", [128, NCH], dt.int32), din("colidx1", [128, NCH], dt.int32)]
    lrcol_t = [din("lrcol0", [128, NCH], dt.int32), din("lrcol1", [128, NCH], dt.int32)]
    lrrow_t = [din("lrrow0", [1, 128 * NCH]), din("lrrow1", [1, 128 * NCH])]
    pidx = din("pidx", [4, 1], dt.int32)
    valrow = din("valrow", [1, 4])
    v_emb = din("v_emb", [VOCAB + 1, H], dt.float32r)
    size_emb = din("size_emb", [26, H], dt.float32r)
    fW1 = din("fW1", [F, H], dt.float32r); fW2 = din("fW2", [H, H], dt.float32r)
    pW1 = din("pW1", [3 * H, 3 * H], dt.float32r); pW2 = din("pW2", [3 * H, H], dt.float32r); pW3 = din("pW3", [H, H], dt.float32r)
    fb1 = din("fb1", [128, 2]); fb2 = din("fb2", [128, 2])
    pb1 = din("pb1", [128, 6]); pb2 = din("pb2", [128, 2]); pb3 = din("pb3", [128, 2])
    We1 = din("We1", [9, 2 * H + 2, H], dt.float32r); We2 = din("We2", [9, H, H], dt.float32r)
    Wn1 = din("Wn1", [9, 2 * H, H], dt.float32r); Wn2 = din("Wn2", [9, H, H], dt.float32r); Wc1 = din("Wc1", [9, H, H], dt.float32r)
    be1 = din("be1", [9, 128, 2]); be2 = din("be2", [9, 128, 2])
    bn1 = din("bn1", [9, 128, 2]); bn2 = din("bn2", [9, 128, 2])
    bc1row = din("bc1row", [9, 1, H])
    Wattrow = din("Wattrow", [9, 1, H]); battrow = din("battrow", [9, 128, 1])
    Wc2row = din("Wc2row", [9, 1, H])
    oW1 = din("oW1", [H + 1, H]); oW2 = din("oW2", [H, VOCAB])
    ob1 = din("ob1", [128, 2]); ob2 = din("ob2", [128, 7])
    ones_row = din("ones_row", [1, 128])
    iota_col = din("iota_col", [128, 1])
    iota_col128 = din("iota_col128", [128, 1])
    iota_row256 = din("iota_row256", [128, 256], dt.int32)

    head_out = nc.dram_tensor("head_out", [4, VOCAB], dt.float32, kind="ExternalOutput")
    dbg_out = {}
    for name in dbg:
        dbg_out[name] = nc.dram_tensor(f"dbg_{name}", [128, G * ROW], dt.float32,
                                       kind="ExternalOutput")

    with tile.TileContext(nc) as tc:
        import contextlib
        ctx = contextlib.ExitStack()
        with ctx:
            pers = ctx.enter_context(tc.tile_pool(name="pers", bufs=1))
            sb = ctx.enter_context(tc.tile_pool(name="sb", bufs=2))
            ps = ctx.enter_context(tc.tile_pool(name="ps", bufs=4, space="PSUM"))
            psacc = ctx.enter_context(tc.tile_pool(name="psacc", bufs=4, space="PSUM"))
            dram = ctx.enter_context(tc.tile_pool(name="dram", bufs=1, space="DRAM"))

            bounce = dram.tile([128, G, ROWB], dt.bfloat16)

            hxnode = pers.tile([128, G, ROW], dt.float32)
            aggT = [pers.tile([128, NS], dt.float32r, tag=f"aggT{i}", name=f"aggT{i}") for i in range(2)]
            xacc = pers.tile([128, G, 4], dt.float32)
            hxb = pers.tile([128, G, ROWB], dt.bfloat16)
            bouncef = dram.tile([128, G, ROW], dt.float32, tag="bouncef", name="bouncef")
            ident = pers.tile([128, 128], dt.float32)
            make_identity(nc, ident[:])
            identb = pers.tile([128, 128], dt.bfloat16)
            nc.vector.tensor_copy(identb[:], ident[:])
            identr = pers.tile([128, 128], dt.float32r)
            nc.vector.tensor_copy(identr[:], ident[:])

            onesr = pers.tile([1, 128], dt.float32)
            nc.sync.dma_start(onesr[:], ones_row[:])
            iotac = pers.tile([128, 1], dt.float32)
            nc.sync.dma_start(iotac[:], iota_col[:])
            iotac128 = pers.tile([128, 1], dt.float32)
            nc.sync.dma_start(iotac128[:], iota_col128[:])
            iotar = pers.tile([128, 256], dt.int32)
            nc.sync.dma_start(iotar[:], iota_row256[:])
            vidxt = pers.tile([128, G], dt.int32)
            nc.sync.dma_start(vidxt[:], v_idx[:])
            sidxt = pers.tile([128, G], dt.int32)
            nc.sync.dma_start(sidxt[:], s_idx[:])

            def mm(out, lhsT, rhs, start, stop):
                nc.tensor.matmul(out=out, lhsT=lhsT, rhs=rhs, start=start, stop=stop)

            def act(out, in_, func, bias=0.0, scale=1.0):
                nc.scalar.activation(out, in_, func, bias=bias, scale=scale)

            # ============ embedding ============
            with tc.tile_pool(name="embw", bufs=1) as embw, \
                 tc.tile_pool(name="embs", bufs=1) as embs:
                xtmp = embs.tile([128, G * 4], dt.float32, tag="xtmp", name="xtmp")
                nc.sync.dma_start(xtmp[:], pos_pm[:])
                nc.vector.tensor_copy(hxnode[:, :, 256:260],
                                      xtmp[:].rearrange("p (g m) -> p g m", m=4))
                nc.vector.tensor_copy(
                    hxb[:, :, 256:264],
                    xtmp[:].rearrange("p (g m) -> p g m", m=4).bitcast(dt.bfloat16))
                def loadw(pool, src, kch, m_, tag):
                    t = pool.tile([128, kch, m_], dt.float32r, tag=tag, name=tag)
                    nc.sync.dma_start(t[:], src[:].rearrange("(k p) m -> p k m", p=128))
                    return t

                fW1t = loadw(embw, fW1, 4, H, "fW1")
                fW2t = loadw(embw, fW2, 2, H, "fW2")
                pW1t = loadw(embw, pW1, 6, 3 * H, "pW1")
                pW2t = loadw(embw, pW2, 6, H, "pW2")
                pW3t = loadw(embw, pW3, 2, H, "pW3")
                bt = {}
                for nm, src, w in (("fb1", fb1, 2), ("fb2", fb2, 2), ("pb1", pb1, 6),
                                   ("pb2", pb2, 2), ("pb3", pb3, 2)):
                    bt[nm] = embw.tile([128, w], dt.float32, tag=nm, name=nm)
                    nc.sync.dma_start(bt[nm][:], src[:])

                for b in range(8):
                    bsl = slice(b * 512, (b + 1) * 512)
                    fe1p = [psacc.tile([128, 512], dt.float32, tag="acc", name="acc") for _ in range(2)]
                    for k in range(4):
                        ft = embs.tile([128, 512], dt.float32r, tag="ft", name="ft")
                        nc.sync.dma_start(ft[:], featT[k * 128:(k + 1) * 128, bsl])
                        for m_ in range(2):
                            mm(fe1p[m_][:], fW1t[:, k, m_ * 128:(m_ + 1) * 128], ft[:],
                               k == 0, k == 3)
                    fe1 = [embs.tile([128, 512], dt.float32r, tag=f"fe1_{i}", name=f"fe1_{i}") for i in range(2)]
                    for m_ in range(2):
                        act(fe1[m_][:], fe1p[m_][:], AF.Silu, bias=bt["fb1"][:, m_:m_ + 1])
                    fe2p = [psacc.tile([128, 512], dt.float32, tag="acc", name="acc") for _ in range(2)]
                    for k in range(2):
                        for m_ in range(2):
                            mm(fe2p[m_][:], fW2t[:, k, m_ * 128:(m_ + 1) * 128], fe1[k][:],
                               k == 0, k == 1)
                    comb = [embs.tile([128, 512], dt.float32r, tag=f"comb{i}", name=f"comb{i}") for i in range(6)]
                    for m_ in range(2):
                        act(comb[2 + m_][:], fe2p[m_][:], AF.Identity,
                            bias=bt["fb2"][:, m_:m_ + 1])
                    for idxt, off in ((vidxt, 0), (sidxt, 4)):
                        tbl = v_emb if off == 0 else size_emb
                        for j in range(4):
                            g = b * 4 + j
                            gt = embs.tile([128, H], dt.float32r, tag="embrow", name="embrow")
                            nc.gpsimd.indirect_dma_start(
                                out=gt[:], out_offset=None, in_=tbl[:],
                                in_offset=bass.IndirectOffsetOnAxis(
                                    ap=idxt[:, g:g + 1], axis=0))
                            for m_ in range(2):
                                tp = ps.tile([128, 128], dt.float32r, tag="small", name="small")
                                nc.tensor.transpose(out=tp[:],
                                                    in_=gt[:, m_ * 128:(m_ + 1) * 128],
                                                    identity=identr[:])
                                dst = comb[(0 if off == 0 else 4) + m_]
                                nc.any.tensor_copy(dst[:, j * 128:(j + 1) * 128], tp[:])
                    hp2p = [psacc.tile([128, 512], dt.float32, tag="acc", name="acc") for _ in range(2)]
                    for mo in range(6):
                        hp1p = psacc.tile([128, 512], dt.float32, tag="acc", name="acc")
                        for k in range(6):
                            mm(hp1p[:], pW1t[:, k, mo * 128:(mo + 1) * 128],
                               comb[k][:], k == 0, k == 5)
                        hp1t = embs.tile([128, 512], dt.float32r, tag="hp1t", name="hp1t")
                        act(hp1t[:], hp1p[:], AF.Silu, bias=bt["pb1"][:, mo:mo + 1])
                        for m_ in range(2):
                            mm(hp2p[m_][:], pW2t[:, mo, m_ * 128:(m_ + 1) * 128], hp1t[:],
                               mo == 0, mo == 5)
                    hp2 = [embs.tile([128, 512], dt.float32r, tag=f"hp2_{i}", name=f"hp2_{i}") for i in range(2)]
                    for m_ in range(2):
                        act(hp2[m_][:], hp2p[m_][:], AF.Silu, bias=bt["pb2"][:, m_:m_ + 1])
                    h0p = [psacc.tile([128, 512], dt.float32, tag="acc", name="acc") for _ in range(2)]
                    for k in range(2):
                        for m_ in range(2):
                            mm(h0p[m_][:], pW3t[:, k, m_ * 128:(m_ + 1) * 128], hp2[k][:],
                               k == 0, k == 1)
                    for m_ in range(2):
                        h0t = embs.tile([128, 512], dt.float32, tag="h0t", name="h0t")
                        act(h0t[:], h0p[m_][:], AF.Identity,
                            bias=bt["pb3"][:, m_:m_ + 1])
                        for j in range(4):
                            g = b * 4 + j
                            tp = ps.tile([128, 128], dt.float32, tag="small", name="small")
                            nc.tensor.transpose(out=tp[:],
                                                in_=h0t[:, j * 128:(j + 1) * 128],
                                                identity=ident[:])
                            nc.any.tensor_copy(hxnode[:, g, m_ * 128:(m_ + 1) * 128], tp[:])
                            nc.any.tensor_copy(hxb[:, g, m_ * 128:(m_ + 1) * 128], tp[:])

            # ============ GCL layers ============
            wpool = ctx.enter_context(tc.tile_pool(name="wpool", bufs=1))
            sb2 = ctx.enter_context(tc.tile_pool(name="sb2", bufs=1))
            selp = ctx.enter_context(tc.tile_pool(name="selp", bufs=8))
            for l in range(nl):
                cfg = 0 if (l // 3) % 2 == 0 else 1
                bases = BASES[cfg]

                nc.sync.dma_start(bounce[:], hxb[:])
                if f"h{l}" in dbg_out:
                    nc.sync.dma_start(
                        dbg_out[f"h{l}"][:].rearrange("p (g m) -> p g m", m=ROW),
                        hxnode[:])
                if sim1:
                    hx_full = dram.tile([NC * 128, G, ROWB], dt.bfloat16,
                                        tag="hxsim", name="hxsim")
                    nc.sync.dma_start(hx_full[0:128, :, :], bounce[:])
                else:
                    hx_full = dram.tile([NC * 128, G, ROWB], dt.bfloat16,
                                        addr_space="Shared", tag=f"hx{l}", name=f"hx{l}")
                nc_ = None
                hx_rows = hx_full[:].rearrange("p g m -> (p g) m")
                if not sim1:
                    nc.gpsimd.collective_compute(
                        "AllGather", mybir.AluOpType.bypass,
                        replica_groups=[list(range(NC))],
                        ins=[bounce.opt()], outs=[hx_full.opt()])

                We1t = wpool.tile([128, 4, H], dt.float32r, tag="We1", name="We1")
                nc.sync.dma_start(We1t[:], We1[l, 0:512, :].rearrange("(k p) m -> p k m", p=128))
                We1r = wpool.tile([2, H], dt.float32r, tag="We1r", name="We1r")
                nc.sync.dma_start(We1r[:], We1[l, 512:514, :])
                We2t = wpool.tile([128, 2, H], dt.float32r, tag="We2", name="We2")
                nc.sync.dma_start(We2t[:], We2[l][:].rearrange("(k p) m -> p k m", p=128))
                Wn1t = wpool.tile([128, 4, H], dt.float32r, tag="Wn1", name="Wn1")
                nc.sync.dma_start(Wn1t[:], Wn1[l][:].rearrange("(k p) m -> p k m", p=128))
                Wn2t = wpool.tile([128, 2, H], dt.float32r, tag="Wn2", name="Wn2")
                nc.sync.dma_start(Wn2t[:], Wn2[l][:].rearrange("(k p) m -> p k m", p=128))
                Wc1t = wpool.tile([128, 2, H], dt.float32r, tag="Wc1", name="Wc1")
                nc.sync.dma_start(Wc1t[:], Wc1[l][:].rearrange("(k p) m -> p k m", p=128))
                lb = {}
                for nm, src in (("be1", be1), ("be2", be2), ("bn1", bn1), ("bn2", bn2)):
                    lb[nm] = wpool.tile([128, 2], dt.float32, tag=f"l{nm}", name=f"l{nm}")
                    nc.sync.dma_start(lb[nm][:], src[l][:])
                battt = wpool.tile([128, 1], dt.float32, tag="batt", name="batt")
                nc.sync.dma_start(battt[:], battrow[l][:])

                def bcast_row(src, tag):
                    r_ = wpool.tile([1, H], dt.float32, tag=tag + "r", name=tag + "r")
                    nc.sync.dma_start(r_[:], src[l][:])
                    p_ = ps.tile([128, H], dt.float32, tag="small", name="small")
                    mm(p_[:], onesr[:], r_[:], True, True)
                    t_ = wpool.tile([128, H], dt.float32, tag=tag, name=tag)
                    nc.any.tensor_copy(t_[:], p_[:])
                    return t_

                wattb = bcast_row(Wattrow, "wattb")
                wc2b = bcast_row(Wc2row, "wc2b")
                bc1b = bcast_row(bc1row, "bc1b")

                colt = sb2.tile([128, NCH], dt.int32, tag="colt", name="colt")
                nc.sync.dma_start(colt[:], colidx_t[cfg][:])
                lrct = sb2.tile([128, NCH], dt.int32, tag="lrct", name="lrct")
                nc.sync.dma_start(lrct[:], lrcol_t[cfg][:])

                nc.gpsimd.memset(aggT[0][:].bitcast(dt.float32), 0.0)
                nc.gpsimd.memset(aggT[1][:].bitcast(dt.float32), 0.0)
                nc.gpsimd.memset(xacc[:], 0.0)

                NST = (NCH + 3) // 4
                for st in range(NST):
                    ch0 = st * 4
                    nch_st = min(4, NCH - ch0)
                    W = nch_st * 128
                    efT = [sb2.tile([128, 512], dt.float32r, tag=f"efT{i}", name=f"efT{i}") for i in range(4)]
                    efr = sb2.tile([2, 512], dt.float32r, tag="efr", name="efr")
                    lrrst = sb.tile([1, 512], dt.float32, tag="lrrst", name="lrrst")
                    nc.sync.dma_start(lrrst[:, :W], lrrow_t[cfg][0:1, ch0 * 128:ch0 * 128 + W])
                    lrbst = ps.tile([128, 512], dt.float32, tag="small", name="small")
                    mm(lrbst[:, :W], onesr[:], lrrst[:, :W], True, True)
                    cd_e = []
                    sel_list = []
                    for j in range(nch_st):
                        k = ch0 + j
                        gbase = bases[k]
                        jsl = slice(j * 128, (j + 1) * 128)
                        cg = sb.tile([128, ROWB], dt.bfloat16, tag="cg", name="cg", bufs=3)
                        nc.gpsimd.indirect_dma_start(
                            out=cg[:], out_offset=None, in_=hx_rows,
                            in_offset=bass.IndirectOffsetOnAxis(
                                ap=colt[:, k:k + 1], axis=0))
                        for m_ in range(2):
                            tpb = ps.tile([128, 128], dt.bfloat16, tag="small", name="small")
                            nc.tensor.transpose(
                                out=tpb[:], in_=cg[:, m_ * 128:(m_ + 1) * 128],
                                identity=identb[:])
                            nc.any.tensor_copy(efT[2 + m_][:, jsl], tpb[:])
                        selT0 = sb.tile([128, 128], dt.float32, tag="selT0", name="selT0")
                        nc.vector.tensor_tensor(
                            out=selT0[:], in0=iotac[:, 0:1].to_broadcast([128, 128]),
                            in1=lrbst[:, jsl], op=ALU.is_equal)
                        selT1 = sb.tile([128, 128], dt.float32, tag="selT1", name="selT1")
                        nc.vector.tensor_tensor(
                            out=selT1[:], in0=iotac128[:, 0:1].to_broadcast([128, 128]),
                            in1=lrbst[:, jsl], op=ALU.is_equal)
                        sel = selp.tile([128, 256], dt.float32r, tag="sel", name="sel")
                        nc.vector.tensor_tensor(
                            out=sel[:], in0=lrct[:, k:k + 1].to_broadcast([128, 256]),
                            in1=iotar[:], op=ALU.is_equal)
                        sel_list.append((sel, gbase))
                        for m_ in range(2):
                            rp = ps.tile([128, 128], dt.float32, tag="small", name="small")
                            for hh, sT in ((0, selT0), (1, selT1)):
                                mm(rp[:], hxnode[:, gbase + hh, m_ * 128:(m_ + 1) * 128],
                                   sT[:], hh == 0, hh == 1)
                            nc.any.tensor_copy(efT[m_][:, jsl], rp[:])
                        xrp = ps.tile([128, 4], dt.float32, tag="small", name="small")
                        for hh, sT in ((0, selT0), (1, selT1)):
                            mm(xrp[:], sT[:], hxnode[:, gbase + hh, 256:260],
                               hh == 0, hh == 1)
                        diff = sb.tile([128, 4], dt.float32, tag="diff", name="diff")
                        nc.vector.tensor_tensor(out=diff[:], in0=xrp[:],
                                                in1=cg[:, 256:264].bitcast(dt.float32),
                                                op=ALU.subtract)
                        sq = sb.tile([128, 3], dt.float32, tag="sq", name="sq")
                        nc.vector.tensor_tensor(out=sq[:], in0=diff[:, 0:3],
                                                in1=diff[:, 0:3], op=ALU.mult)
                        rad = sb.tile([128, 1], dt.float32, tag="rad", name="rad")
                        nc.vector.tensor_reduce(out=rad[:], in_=sq[:],
                                                axis=mybir.AxisListType.X, op=ALU.add)
                        den = sb.tile([128, 1], dt.float32, tag="den", name="den")
                        act(den[:], rad[:], AF.Sqrt)
                        nc.vector.tensor_scalar_add(out=den[:], in0=den[:], scalar1=1.0)
                        rec = sb.tile([128, 1], dt.float32, tag="rec", name="rec")
                        nc.vector.reciprocal(rec[:], den[:])
                        cd = selp.tile([128, 4], dt.float32, tag="cd", name="cd")
                        nc.vector.tensor_scalar_mul(out=cd[:], in0=diff[:], scalar1=rec[:, 0:1])
                        cd_e.append(cd)
                        rad2 = sb.tile([128, 2], dt.float32, tag="rad2", name="rad2")
                        nc.vector.tensor_copy(rad2[:], rad[:, 0:1].to_broadcast([128, 2]))
                        radT = ps.tile([2, 128], dt.float32, tag="small", name="small")
                        nc.tensor.transpose(out=radT[:], in_=rad2[:], identity=ident[:])
                        nc.any.tensor_copy(efr[0:2, jsl], radT[:])

                    m1p = [psacc.tile([128, 512], dt.float32, tag="acc", name="acc") for _ in range(2)]
                    for m_ in range(2):
                        msl = slice(m_ * 128, (m_ + 1) * 128)
                        for k in range(4):
                            mm(m1p[m_][:, :W], We1t[:, k, msl], efT[k][:, :W], k == 0, False)
                        mm(m1p[m_][:, :W], We1r[:, msl], efr[:, :W], False, True)
                    msg1 = [sb2.tile([128, 512], dt.float32r, tag=f"msg1_{i}", name=f"msg1_{i}") for i in range(2)]
                    for m_ in range(2):
                        act(msg1[m_][:, :W], m1p[m_][:, :W], AF.Silu,
                            bias=lb["be1"][:, m_:m_ + 1])
                    m2p = [psacc.tile([128, 512], dt.float32, tag="acc", name="acc") for _ in range(2)]
                    for m_ in range(2):
                        msl = slice(m_ * 128, (m_ + 1) * 128)
                        for k in range(2):
                            mm(m2p[m_][:, :W], We2t[:, k, msl], msg1[k][:, :W], k == 0, k == 1)
                    msg2 = [sb2.tile([128, 512], dt.float32r, tag=f"msg2_{i}", name=f"msg2_{i}") for i in range(2)]
                    for m_ in range(2):
                        act(msg2[m_][:, :W], m2p[m_][:, :W], AF.Silu,
                            bias=lb["be2"][:, m_:m_ + 1])

                    for j in range(nch_st):
                        k = ch0 + j
                        sel, gbase = sel_list[j]
                        jsl = slice(j * 128, (j + 1) * 128)
                        m2e = sb.tile([128, 256], dt.float32, tag="m2e", name="m2e")
                        for m_ in range(2):
                            tpr = ps.tile([128, 128], dt.float32r, tag="small", name="small")
                            nc.tensor.transpose(out=tpr[:], in_=msg2[m_][:, jsl],
                                                identity=identr[:])
                            nc.any.tensor_copy(m2e[:, m_ * 128:(m_ + 1) * 128], tpr[:])
                        am = sb.tile([128, 256], dt.float32, tag="scr256", name="am")
                        nc.vector.tensor_tensor(out=am[:], in0=m2e[:], in1=wattb[:],
                                                op=ALU.mult)
                        att = sb.tile([128, 1], dt.float32, tag="att", name="att")
                        nc.vector.tensor_reduce(out=att[:], in_=am[:],
                                                axis=mybir.AxisListType.X, op=ALU.add)
                        atts = sb.tile([128, 1], dt.float32, tag="atts", name="atts")
                        act(atts[:], att[:], AF.Sigmoid, bias=battt[:, 0:1])
                        msge = sb.tile([128, 256], dt.float32r, tag="msge", name="msge")
                        nc.vector.tensor_scalar_mul(out=msge[:], in0=m2e[:],
                                                    scalar1=atts[:, 0:1])
                        for m_ in range(2):
                            sp = ps.tile([128, 256], dt.float32, tag="small", name="small")
                            mm(sp[:], msge[:, m_ * 128:(m_ + 1) * 128], sel[:], True, True)
                            nc.vector.tensor_tensor(
                                out=aggT[m_][:, gbase * 128:gbase * 128 + 256],
                                in0=aggT[m_][:, gbase * 128:gbase * 128 + 256],
                                in1=sp[:], op=ALU.add)
                        we = ps.tile([128, 256], dt.float32, tag="small", name="small")
                        for fc in range(2):
                            mm(we[:], msg2[fc][:, jsl], Wc1t[:, fc, :], fc == 0, fc == 1)
                        u = sb.tile([128, 256], dt.float32, tag="u", name="u")
                        nc.vector.tensor_scalar_mul(out=u[:], in0=we[:], scalar1=atts[:, 0:1])
                        nc.vector.tensor_tensor(out=u[:], in0=u[:], in1=bc1b[:], op=ALU.add)
                        te = sb.tile([128, 256], dt.float32, tag="te", name="te")
                        act(te[:], u[:], AF.Silu)
                        zm = sb.tile([128, 256], dt.float32, tag="scr256", name="zm")
                        nc.vector.tensor_tensor(out=zm[:], in0=te[:], in1=wc2b[:], op=ALU.mult)
                        z = sb.tile([128, 1], dt.float32, tag="z", name="z")
                        nc.vector.tensor_reduce(out=z[:], in_=zm[:],
                                                axis=mybir.AxisListType.X, op=ALU.add)
                        th = sb.tile([128, 1], dt.float32, tag="th", name="th")
                        act(th[:], z[:], AF.Tanh)
                        nc.vector.tensor_scalar_mul(out=th[:], in0=th[:],
                                                    scalar1=float(COORDS_RANGE))
                        cdt = sb.tile([128, 4], dt.float32r, tag="cdt", name="cdt")
                        nc.vector.tensor_scalar_mul(out=cdt[:], in0=cd_e[j][:],
                                                    scalar1=th[:, 0:1])
                        for hh in range(2):
                            xp = ps.tile([128, 4], dt.float32, tag="small", name="small")
                            mm(xp[:], sel[:, hh * 128:(hh + 1) * 128], cdt[:], True, True)
                            nc.vector.tensor_tensor(out=xacc[:, gbase + hh, :],
                                                    in0=xacc[:, gbase + hh, :],
                                                    in1=xp[:], op=ALU.add)

                for g in range(G):
                    nc.vector.tensor_tensor(out=hxnode[:, g, 256:260],
                                            in0=hxnode[:, g, 256:260],
                                            in1=xacc[:, g, :], op=ALU.add)
                nc.vector.tensor_copy(hxb[:, :, 256:264],
                                      hxnode[:, :, 256:260].bitcast(dt.bfloat16))

                for b in range(8):
                    bsl = slice(b * 512, (b + 1) * 512)
                    hTb = sb2.tile([128, 2, 512], dt.float32, tag="hTb", name="hTb")
                    for m_ in range(2):
                        for j in range(4):
                            g = b * 4 + j
                            tp = ps.tile([128, 128], dt.float32, tag="small", name="small")
                            nc.tensor.transpose(out=tp[:],
                                                in_=hxnode[:, g, m_ * 128:(m_ + 1) * 128],
                                                identity=ident[:])
                            nc.any.tensor_copy(hTb[:, m_, j * 128:(j + 1) * 128], tp[:])
                    hTbr = sb2.tile([128, 2, 512], dt.float32r, tag="hTbr", name="hTbr")
                    nc.any.tensor_copy(hTbr[:], hTb[:])
                    n1p = [psacc.tile([128, 512], dt.float32, tag="acc", name="acc") for _ in range(2)]
                    for m_ in range(2):
                        msl = slice(m_ * 128, (m_ + 1) * 128)
                        for k in range(2):
                            mm(n1p[m_][:], Wn1t[:, k, msl], hTbr[:, k, :], k == 0, False)
                        for k in range(2):
                            mm(n1p[m_][:], Wn1t[:, 2 + k, msl], aggT[k][:, bsl],
                               False, k == 1)
                    nh1 = [sb2.tile([128, 512], dt.float32r, tag=f"nh1_{i}", name=f"nh1_{i}") for i in range(2)]
                    for m_ in range(2):
                        act(nh1[m_][:], n1p[m_][:], AF.Silu, bias=lb["bn1"][:, m_:m_ + 1])
                    n2p = [psacc.tile([128, 512], dt.float32, tag="acc", name="acc") for _ in range(2)]
                    for m_ in range(2):
                        msl = slice(m_ * 128, (m_ + 1) * 128)
                        for k in range(2):
                            mm(n2p[m_][:], Wn2t[:, k, msl], nh1[k][:], k == 0, k == 1)
                    for m_ in range(2):
                        nh2 = sb.tile([128, 512], dt.float32, tag="nh2", name="nh2")
                        act(nh2[:], n2p[m_][:], AF.Identity, bias=lb["bn2"][:, m_:m_ + 1])
                        nc.vector.tensor_tensor(out=hTb[:, m_, :], in0=hTb[:, m_, :],
                                                in1=nh2[:], op=ALU.add)
                        for j in range(4):
                            g = b * 4 + j
                            tp = ps.tile([128, 128], dt.float32, tag="small", name="small")
                            nc.tensor.transpose(out=tp[:],
                                                in_=hTb[:, m_, j * 128:(j + 1) * 128],
                                                identity=ident[:])
                            nc.any.tensor_copy(hxnode[:, g, m_ * 128:(m_ + 1) * 128], tp[:])
                            nc.any.tensor_copy(hxb[:, g, m_ * 128:(m_ + 1) * 128], tp[:])

            nc.sync.dma_start(bouncef[:], hxnode[:])
            if f"h{nl}" in dbg_out:
                nc.sync.dma_start(
                    dbg_out[f"h{nl}"][:].rearrange("p (g m) -> p g m", m=ROW), hxnode[:])
            bounce_rows = bouncef[:].rearrange("p g m -> (p g) m")

            # ============ output head ============
            if with_head:
                oW1t = wpool.tile([128, 2, H], dt.float32, tag="oW1", name="oW1")
                nc.sync.dma_start(oW1t[:], oW1[0:256, :].rearrange("(k p) m -> p k m", p=128))
                oW1v = wpool.tile([1, H], dt.float32, tag="oW1v", name="oW1v")
                nc.sync.dma_start(oW1v[:], oW1[256:257, :])
                oW2t = wpool.tile([128, 2, VOCAB], dt.float32, tag="oW2", name="oW2")
                nc.sync.dma_start(oW2t[:], oW2[:].rearrange("(k p) m -> p k m", p=128))
                ob1t = wpool.tile([128, 2], dt.float32, tag="ob1", name="ob1")
                nc.sync.dma_start(ob1t[:], ob1[:])
                ob2t = wpool.tile([128, 7], dt.float32, tag="ob2", name="ob2")
                nc.sync.dma_start(ob2t[:], ob2[:])
                pidxt = sb.tile([4, 1], dt.int32, tag="pidxt", name="pidxt")
                nc.sync.dma_start(pidxt[:], pidx[:])
                valt = sb.tile([1, 4], dt.float32, tag="valt", name="valt")
                nc.sync.dma_start(valt[:], valrow[:])

                hsel = sb.tile([4, ROW], dt.float32, tag="hsel", name="hsel")
                nc.gpsimd.indirect_dma_start(
                    out=hsel[:], out_offset=None, in_=bounce_rows,
                    in_offset=bass.IndirectOffsetOnAxis(ap=pidxt[:, :1], axis=0))
                hselT = sb.tile([128, 2, 4], dt.float32, tag="hselT", name="hselT")
                for m_ in range(2):
                    tp = ps.tile([128, 4], dt.float32, tag="small", name="small")
                    nc.tensor.transpose(out=tp[:], in_=hsel[:, m_ * 128:(m_ + 1) * 128],
                                        identity=ident[:4, :4])
                    nc.any.tensor_copy(hselT[:, m_, :], tp[:])
                o1p = [ps.tile([128, 4], dt.float32, tag="small", name="small") for _ in range(2)]
                for m_ in range(2):
                    msl = slice(m_ * 128, (m_ + 1) * 128)
                    for k in range(2):
                        mm(o1p[m_][:], oW1t[:, k, msl], hselT[:, k, :], k == 0, False)
                    mm(o1p[m_][:], oW1v[:, msl], valt[:], False, True)
                o1 = sb.tile([128, 2, 4], dt.float32, tag="o1", name="o1")
                for m_ in range(2):
                    act(o1[:, m_, :], o1p[m_][:], AF.Silu, bias=ob1t[:, m_:m_ + 1])
                hout = sb.tile([4, VOCAB], dt.float32, tag="hout", name="hout")
                for mo in range(7):
                    mw = min(128, VOCAB - mo * 128)
                    o2p = ps.tile([128, 4], dt.float32, tag="small", name="small")
                    for k in range(2):
                        mm(o2p[:mw, :], oW2t[:, k, mo * 128:mo * 128 + mw], o1[:, k, :],
                           k == 0, k == 1)
                    o2b = sb.tile([128, 4], dt.float32, tag="o2b", name="o2b")
                    act(o2b[:mw, :], o2p[:mw, :], AF.Identity, bias=ob2t[:mw, mo:mo + 1])
                    fp = ps.tile([4, 128], dt.float32, tag="small", name="small")
                    nc.tensor.transpose(out=fp[:, :mw], in_=o2b[:mw, :],
                                        identity=ident[:mw, :mw])
                    nc.any.tensor_copy(hout[:, mo * 128:mo * 128 + mw], fp[:, :mw])
                nc.sync.dma_start(head_out[:], hout[:])
            else:
                zt = sb.tile([4, VOCAB], dt.float32, tag="zt", name="zt")
                nc.gpsimd.memset(zt[:], 0.0)
                nc.sync.dma_start(head_out[:], zt[:])

    nc.compile()
    return nc


def build_and_run(inputs, nl=N_LAYERS, with_head=True, dbg=(), trace=False):
    from concourse.bass_utils import run_bass_kernel_spmd
    meta, maps = _prep(inputs)
    key = (nl, with_head, tuple(dbg))
    if key not in _cache:
        _cache[key] = _build(meta, nl=nl, with_head=with_head, dbg=dbg)
    nc = _cache[key]
    res = run_bass_kernel_spmd(nc, maps, core_ids=list(range(NC)), trace=trace)
    return res


def decode_state(arr):
    """dbg [128, G*ROW] p-major -> (h [4096, 256], x [4096, 3])"""
    a = arr.reshape(128, G, ROW).transpose(1, 0, 2).reshape(NS, ROW)
    return a[:, :256], a[:, 256:259]


def kernel(**inputs) -> np.ndarray:
    res = build_and_run(inputs)
    out = np.concatenate([res.results[c]["head_out"] for c in range(NC)], 0)
    return out.astype(np.float32)


def timed_run(inputs, iters=10, nl=N_LAYERS):
    """Time repeated on-device executions (min wall per exec, ns)."""
    import time
    import jax
    from jax.experimental.shard_map import shard_map
    from jax.sharding import Mesh, PartitionSpec
    from concourse import bass2jax, mybir

    meta, maps = _prep(inputs)
    key = (nl, True, ())
    if key not in _cache:
        _cache[key] = _build(meta, nl=nl, with_head=True, dbg=())
    nc = _cache[key]
    bass2jax.install_neuronx_cc_hook()

    in_names, out_names, out_avals, zero_outs = [], [], [], []
    partition_name = nc.partition_id_tensor.name if nc.partition_id_tensor else None
    for alloc in nc.m.functions[0].allocations:
        if not isinstance(alloc, bass2jax.mybir.MemoryLocationSet):
            continue
        name = alloc.memorylocations[0].name
        if alloc.kind == "ExternalInput":
            if name != partition_name:
                in_names.append(name)
        elif alloc.kind == "ExternalOutput":
            shape = tuple(alloc.tensor_shape)
            dtype = mybir.dt.np(alloc.dtype)
            out_avals.append(jax.core.ShapedArray(shape, dtype))
            out_names.append(name)
            zero_outs.append(np.zeros(shape, dtype))
    n_params = len(in_names)
    all_in = in_names + out_names + ([partition_name] if partition_name else [])

    def _body(*args):
        operands = list(args)
        if partition_name is not None:
            operands.append(bass2jax.partition_id_tensor())
        outs = bass2jax._bass_exec_p.bind(
            *operands, out_avals=tuple(out_avals), in_names=tuple(all_in),
            out_names=tuple(out_names), lowering_input_output_aliases=(),
            sim_require_finite=True, sim_require_nnan=True, nc=nc)
        return tuple(outs)

    devices = jax.devices()[:NC]
    mesh = Mesh(np.asarray(devices), ("core",))
    nin = n_params + len(out_names)
    fn = jax.jit(shard_map(_body, mesh=mesh,
                           in_specs=(PartitionSpec("core"),) * nin,
                           out_specs=(PartitionSpec("core"),) * len(out_names),
                           check_rep=False), keep_unused=True)
    concat_in = [np.concatenate([np.asarray(maps[c][nm]) for c in range(NC)], 0)
                 for nm in in_names]
    concat_zero = [np.zeros((NC * z.shape[0], *z.shape[1:]), z.dtype)
                   for z in zero_outs]
    dev_in = [jax.device_put(a) for a in concat_in]
    dev_zero = [jax.device_put(a) for a in concat_zero]
    out = fn(*dev_in, *dev_zero)
    jax.block_until_ready(out)
    times = []
    for _ in range(iters):
        t0 = time.perf_counter()
        out = fn(*dev_in, *dev_zero)
        jax.block_until_ready(out)
        times.append(time.perf_counter() - t0)
    return min(times) * 1e9, times



# revision 2
# speedup vs baseline: 1.0501x; 1.0501x over previous
"""Trainium2 Bass kernel for nn_Node2Vec (EGNN message passing), 8-core SPMD.

v2 design (instruction-count / dependency optimized):
- h master kept FEATURE-major f32 in SBUF (hT [128, 2, 4096]); bf16 shadow hTb.
- Per layer: h+x exchanged via AllGather of node-major DRAM replicas
  (h rows 512B bf16; x rows 16B as bf16 hi/lo split pairs).
- Edge gathers: SWDGE dma_gather in transpose mode -> feature-major ef tiles
  directly (no PE transposes after gathers). Row-side h gathered from the
  LOCAL bounce (overlaps the AllGather).
- Edges packed into 128-slot chunks by 256-node destination windows (shared
  schedule across cores); scatters are bf16 selection-matrix matmuls
  accumulated in PSUM chains per window (no serial DVE adds).
- All MLP matmuls bf16 (1 cyc/row); activations phase-grouped per layer so
  the Act engine loads each function table ~once per layer.
- x radial math edge-major in f32 from an exact bf16 hi/lo split exchange.

DRAM replica row permutation: node local id r -> row (r%128)*32 + r//128
(so SBUF [128, G, *] <-> DRAM rows are contiguous per partition).
"""
import numpy as np
import ml_dtypes

NC = 8
N = 32768
NS = N // NC          # 4096 nodes per core
G = 32                # 128-node groups per core
NW = 16               # 256-node scatter windows per core
H = 256
F = 512
VOCAB = 780
BS = 32
N_LAYERS = 9
COORDS_RANGE = 30.0
EBC = 8               # chunks per efT gather block (2 MLP stages)

bf16 = ml_dtypes.bfloat16
_cache = {}


def _permrow(n):
    """DRAM p-major row index for global node id n."""
    n = np.asarray(n)
    return (n // NS) * NS + (n % NS) % 128 * G + (n % NS) // 128


def _wrap16(ids, nidx):
    """int16 SWDGE idx layout: idx i at partition i%16, col i//16,
    replicated across the 8 gpsimd core groups."""
    ncol = (nidx + 15) // 16
    a = np.zeros(ncol * 16, np.int64)
    a[:len(ids)] = ids
    assert a.max() < 32768 and a.min() >= 0
    w = a.reshape(ncol, 16).T
    return np.ascontiguousarray(np.tile(w, (8, 1)).astype(np.int16))


def _pack(edges):
    """Shared chunk schedule + per-core sel/selT/index data per config."""
    packs = []
    for cfg in (0, 1):
        row = edges[cfg].astype(np.int64)
        col = edges[1 - cfg].astype(np.int64)
        cnt = np.zeros((NC, NW), np.int64)
        percore = []
        for c in range(NC):
            m = (row // NS) == c
            r = row[m] - c * NS
            k = col[m]
            o = np.argsort(r, kind="stable")
            r, k = r[o], k[o]
            percore.append((r, k))
            cnt[c] = np.bincount(r // 256, minlength=NW)
        cpw = np.maximum(np.ceil(cnt.max(0) / 128).astype(int), 1)
        NCH = int(cpw.sum())
        wstart = np.zeros(NW, int)
        wstart[1:] = np.cumsum(cpw)[:-1]
        sel = np.zeros((NC, 128, NCH, 256), np.float32)
        selT = np.zeros((NC, 128, NCH, 256), np.float32)
        rowi = np.zeros((NC, NCH * 128), np.int64)
        coli = np.zeros((NC, NCH * 128), np.int64)
        colx = np.zeros((NC, 128, NCH), np.int32)
        for c in range(NC):
            r, k = percore[c]
            wofall = r // 256
            for w in range(NW):
                idx = np.nonzero(wofall == w)[0]
                for j, e in enumerate(idx):
                    ch = wstart[w] + j // 128
                    sl = j % 128
                    lr = int(r[e] - w * 256)
                    sel[c, sl, ch, lr] = 1
                    selT[c, lr % 128, ch, (lr // 128) * 128 + sl] = 1
                    gi = ch * 128 + sl
                    rowi[c, gi] = (r[e] % 128) * G + r[e] // 128
                    coli[c, gi] = _permrow(k[e])
                    colx[c, sl, ch] = _permrow(k[e])
        packs.append(dict(NCH=NCH, cpw=cpw, wstart=wstart,
                          sel=sel, selT=selT, rowi=rowi, coli=coli, colx=colx))
    return packs


def _prep(inputs):
    f32 = np.float32
    feature = np.asarray(inputs["feature"], f32).reshape(N, F)
    v = np.asarray(inputs["v"]).astype(np.int64).reshape(N)
    size = np.asarray(inputs["size"]).astype(np.int64).reshape(N)
    pos = np.asarray(inputs["pos"], f32).reshape(N, 3)
    edges = np.asarray(inputs["edges"]).astype(np.int64)
    predict_idx = np.asarray(inputs["predict_idx"]).astype(np.int64)
    val = np.asarray(inputs["val"], f32)

    packs = _pack(edges)
    meta = dict(NCH=(packs[0]["NCH"], packs[1]["NCH"]),
                cpw=(tuple(packs[0]["cpw"]), tuple(packs[1]["cpw"])),
                wstart=(tuple(packs[0]["wstart"]), tuple(packs[1]["wstart"])))

    def b(x):
        return np.ascontiguousarray(np.asarray(x, f32).astype(bf16))

    def f(x):
        return np.ascontiguousarray(np.asarray(x, f32))

    def halves(bias, k):
        return f(np.asarray(bias, f32).reshape(k, 128).T)

    We1 = np.asarray(inputs["We1"], f32)   # [9, 514, 256]
    be1 = np.asarray(inputs["be1"], f32)   # [9, 256]
    We1aug = np.zeros((9, 4, 256), f32)
    We1aug[:, 0] = We1[:, 512]
    We1aug[:, 1] = We1[:, 513]
    We1aug[:, 2] = be1
    shared = dict(
        fW1=b(inputs["fW1"]), fW2=b(inputs["fW2"]),
        pW1=b(inputs["pW1"]), pW2=b(inputs["pW2"]), pW3=b(inputs["pW3"]),
        fb1=halves(inputs["fb1"], 2), fb2=halves(inputs["fb2"], 2),
        pb1=halves(inputs["pb1"], 6), pb2=halves(inputs["pb2"], 2),
        pb3=halves(inputs["pb3"], 2),
        v_emb=b(inputs["v_emb"]), size_emb=b(inputs["size_emb"]),
        We1t9=b(We1[:, 0:512, :]), We1a9=b(We1aug),
        We29=b(inputs["We2"]), Wn19=b(inputs["Wn1"]), Wn29=b(inputs["Wn2"]),
        Wc19=b(inputs["Wc1"]),
        wattv9=b(np.asarray(inputs["Watt"], f32).reshape(9, 2, 128).transpose(0, 2, 1)),
        wc2v9=b(np.asarray(inputs["Wc2"], f32).reshape(9, 2, 128).transpose(0, 2, 1)),
        be29=np.stack([halves(np.asarray(inputs["be2"])[l], 2) for l in range(9)]),
        bn19=np.stack([halves(np.asarray(inputs["bn1"])[l], 2) for l in range(9)]),
        bc19=np.stack([halves(np.asarray(inputs["bc1"])[l], 2) for l in range(9)]),
        bn2r9=b(np.asarray(inputs["bn2"], f32).reshape(9, 1, 256)),
        batt9=f(np.asarray(inputs["batt"], f32).reshape(9, 1, 1)),
        oW1=b(np.asarray(inputs["oW1"])[0:256, :]),
        oW1v=b(np.asarray(inputs["oW1"])[256:257, :]),
        oW2=b(inputs["oW2"]),
        ob1=halves(inputs["ob1"], 2),
        ob2=f(np.pad(np.asarray(inputs["ob2"], f32), (0, 128 * 7 - VOCAB)).reshape(7, 128).T),
        ones128=b(np.ones((1, 128))),
        ones512=b(np.ones((1, 512))),
    )

    maps = []
    for c in range(NC):
        sl = slice(c * NS, (c + 1) * NS)
        pos_pm = np.zeros((128, G, 4), f32)
        pos_pm[:, :, :3] = pos[sl].reshape(G, 128, 3).transpose(1, 0, 2)
        nloc = np.arange(4) * 1024 + predict_idx[4 * c:4 * c + 4]
        ploc = ((nloc % 128) * G + nloc // 128).astype(np.int32).reshape(4, 1)
        m = dict(
            featT=b(feature[sl].T),
            pos_pm=f(pos_pm.reshape(128, G * 4)),
            vidx16=_wrap16(v[sl], NS), sidx16=_wrap16(size[sl], NS),
            pidx=ploc,
            valrow=f(val[4 * c:4 * c + 4].reshape(1, 4)),
        )
        for cfg in (0, 1):
            p = packs[cfg]
            NE = p["NCH"] * 128
            m[f"sel{cfg}"] = b(p["sel"][c].reshape(128, -1))
            m[f"selT{cfg}"] = b(p["selT"][c].reshape(128, -1))
            m[f"rowi{cfg}"] = _wrap16(p["rowi"][c], NE)
            m[f"coli{cfg}"] = _wrap16(p["coli"][c], NE)
            m[f"colx{cfg}"] = np.ascontiguousarray(p["colx"][c])
        m.update(shared)
        maps.append(m)
    return meta, maps


def _build(meta, nl=N_LAYERS, with_head=True, dbg=(), sim1=False):
    import concourse.bacc as bacc
    import concourse.bass as bass
    import concourse.mybir as mybir
    import concourse.tile as tile
    from concourse.masks import make_identity

    dt = mybir.dt
    AF = mybir.ActivationFunctionType
    ALU = mybir.AluOpType
    NCH = meta["NCH"]
    CPW = meta["cpw"]
    WSTART = meta["wstart"]
    NCHMX = max(NCH)

    nc = bacc.Bacc("TRN2", target_bir_lowering=False, debug=False,
                   num_devices=1 if sim1 else NC, enable_asserts=False)

    def din(name, shape, d=dt.float32):
        return nc.dram_tensor(name, list(shape), d, kind="ExternalInput")

    featT = din("featT", [F, NS], dt.bfloat16)
    pos_pm = din("pos_pm", [128, G * 4])
    vidx16 = din("vidx16", [128, NS // 16], dt.int16)
    sidx16 = din("sidx16", [128, NS // 16], dt.int16)
    pidx = din("pidx", [4, 1], dt.int32)
    valrow = din("valrow", [1, 4])
    sel_d, selT_d, rowi_d, coli_d, colx_d = [], [], [], [], []
    for cfg in (0, 1):
        ne = NCH[cfg] * 128
        sel_d.append(din(f"sel{cfg}", [128, NCH[cfg] * 256], dt.bfloat16))
        selT_d.append(din(f"selT{cfg}", [128, NCH[cfg] * 256], dt.bfloat16))
        rowi_d.append(din(f"rowi{cfg}", [128, ne // 16], dt.int16))
        coli_d.append(din(f"coli{cfg}", [128, ne // 16], dt.int16))
        colx_d.append(din(f"colx{cfg}", [128, NCH[cfg]], dt.int32))
    fW1 = din("fW1", [F, H], dt.bfloat16)
    fW2 = din("fW2", [H, H], dt.bfloat16)
    pW1 = din("pW1", [3 * H, 3 * H], dt.bfloat16)
    pW2 = din("pW2", [3 * H, H], dt.bfloat16)
    pW3 = din("pW3", [H, H], dt.bfloat16)
    fb1 = din("fb1", [128, 2]); fb2 = din("fb2", [128, 2])
    pb1 = din("pb1", [128, 6]); pb2 = din("pb2", [128, 2]); pb3 = din("pb3", [128, 2])
    v_emb = din("v_emb", [VOCAB + 1, H], dt.bfloat16)
    size_emb = din("size_emb", [26, H], dt.bfloat16)
    We1t9 = din("We1t9", [9, 512, H], dt.bfloat16)
    We1a9 = din("We1a9", [9, 4, H], dt.bfloat16)
    We29 = din("We29", [9, H, H], dt.bfloat16)
    Wn19 = din("Wn19", [9, 2 * H, H], dt.bfloat16)
    Wn29 = din("Wn29", [9, H, H], dt.bfloat16)
    Wc19 = din("Wc19", [9, H, H], dt.bfloat16)
    wattv9 = din("wattv9", [9, 128, 2], dt.bfloat16)
    wc2v9 = din("wc2v9", [9, 128, 2], dt.bfloat16)
    be29 = din("be29", [9, 128, 2]); bn19 = din("bn19", [9, 128, 2])
    bc19 = din("bc19", [9, 128, 2]); bn2r9 = din("bn2r9", [9, 1, H], dt.bfloat16)
    batt9 = din("batt9", [9, 1, 1])
    oW1 = din("oW1", [H, H], dt.bfloat16)
    oW1v = din("oW1v", [1, H], dt.bfloat16)
    oW2 = din("oW2", [H, VOCAB], dt.bfloat16)
    ob1 = din("ob1", [128, 2]); ob2 = din("ob2", [128, 7])
    ones128 = din("ones128", [1, 128], dt.bfloat16)
    ones512 = din("ones512", [1, 512], dt.bfloat16)

    head_out = nc.dram_tensor("head_out", [4, VOCAB], dt.float32, kind="ExternalOutput")
    dbg_out = {}
    for name in dbg:
        dbg_out[name + "h"] = nc.dram_tensor(f"dbg_{name}h", [128, 2 * NS], dt.float32,
                                             kind="ExternalOutput")
        dbg_out[name + "x"] = nc.dram_tensor(f"dbg_{name}x", [128, G * 4], dt.float32,
                                             kind="ExternalOutput")

    with tile.TileContext(nc) as tc:
        import contextlib
        ctx = contextlib.ExitStack()
        with ctx:
            pers = ctx.enter_context(tc.tile_pool(name="pers", bufs=1))
            big = ctx.enter_context(tc.tile_pool(name="big", bufs=1))
            rot = ctx.enter_context(tc.tile_pool(name="rot", bufs=2))
            wp = ctx.enter_context(tc.tile_pool(name="wp", bufs=1))
            psA = ctx.enter_context(tc.tile_pool(name="psA", bufs=2, space="PSUM"))
            psG = ctx.enter_context(tc.tile_pool(name="psG", bufs=1, space="PSUM"))
            psS = ctx.enter_context(tc.tile_pool(name="psS", bufs=2, space="PSUM"))
            dram = ctx.enter_context(tc.tile_pool(name="dram", bufs=1, space="DRAM"))

            bounce_h = dram.tile([128, G, H], dt.bfloat16)
            bounce_x = dram.tile([128, G, 8], dt.bfloat16)

            hT = pers.tile([128, 2, NS], dt.float32)
            xb = pers.tile([128, G, 4], dt.float32)
            xbb2 = pers.tile([128, G, 8], dt.bfloat16)

            ident = pers.tile([128, 128], dt.float32)
            make_identity(nc, ident[:])
            identb = pers.tile([128, 128], dt.bfloat16)
            nc.vector.tensor_copy(identb[:], ident[:])
            ones128t = pers.tile([1, 128], dt.bfloat16)
            nc.sync.dma_start(ones128t[:], ones128[:])
            ones512t = pers.tile([1, 512], dt.bfloat16)
            nc.sync.dma_start(ones512t[:], ones512[:])
            # radial scratch rows [rad, rad, 1, 0] per chunk
            radx = pers.tile([128, NCHMX, 4], dt.bfloat16)
            nc.gpsimd.memset(radx[:], 0.0)
            nc.vector.tensor_scalar_add(out=radx[:, :, 2:3], in0=radx[:, :, 2:3],
                                        scalar1=1.0)

            def mm(out, lhsT, rhs, start, stop):
                nc.tensor.matmul(out=out, lhsT=lhsT, rhs=rhs, start=start, stop=stop)

            def act(out, in_, func, bias=0.0, scale=1.0):
                nc.scalar.activation(out, in_, func, bias=bias, scale=scale)

            # ============ embedding ============
            if True:
                xtmp = rot.tile([128, G * 4], dt.float32, tag="xlo", name="xtmp")
                nc.sync.dma_start(xtmp[:], pos_pm[:])
                nc.vector.tensor_copy(xb[:], xtmp[:].rearrange("p (g m) -> p g m", m=4))

                def loadw(src, kch, m_, tag, pool):
                    t = pool.tile([128, kch, m_], dt.bfloat16, tag=tag, name="ew" + tag)
                    nc.sync.dma_start(t[:], src[:].rearrange("(k p) m -> p k m", p=128))
                    return t

                fW1t = loadw(fW1, 4, H, "We1", wp)
                fW2t = loadw(fW2, 2, H, "We2", wp)
                pW1t = loadw(pW1, 6, 3 * H, "msg2", big)
                pW2t = loadw(pW2, 6, H, "oW2", wp)
                pW3t = loadw(pW3, 2, H, "Wn2", wp)
                bt = {}
                for nm, src, w, tg in (("fb1", fb1, 2, "be2"), ("fb2", fb2, 2, "bn1"),
                                       ("pb1", pb1, 6, "pb1"), ("pb2", pb2, 2, "bc1"),
                                       ("pb3", pb3, 2, "pb3")):
                    bt[nm] = wp.tile([128, w], dt.float32, tag=tg, name="ew" + nm)
                    nc.sync.dma_start(bt[nm][:], src[:])
                vit = wp.tile([128, NS // 16], dt.int16, tag="vit", name="vit")
                nc.sync.dma_start(vit[:], vidx16[:])
                sit = wp.tile([128, NS // 16], dt.int16, tag="sit", name="sit")
                nc.sync.dma_start(sit[:], sidx16[:])

                for b in range(8):
                    bsl = slice(b * 512, (b + 1) * 512)
                    csl = slice(b * 32, (b + 1) * 32)
                    comb = big.tile([128, 6, 512], dt.bfloat16, tag="bigA",
                                    name="comb", bufs=2)
                    nc.gpsimd.dma_gather(
                        out_ap=comb[:, 0:2, :], in_ap=v_emb[:], idxs_ap=vit[:, csl],
                        num_idxs=512, num_idxs_reg=512, elem_size=H, transpose=True)
                    nc.gpsimd.dma_gather(
                        out_ap=comb[:, 4:6, :], in_ap=size_emb[:], idxs_ap=sit[:, csl],
                        num_idxs=512, num_idxs_reg=512, elem_size=H, transpose=True)
                    ft = big.tile([128, 4, 512], dt.bfloat16, tag="bigA", name="ft", bufs=2)
                    nc.sync.dma_start(
                        ft[:], featT[:].rearrange("(k p) n -> p k n", p=128)[:, :, bsl])
                    fe1p = psA.tile([128, 2, 512], dt.float32, tag="accb", name="accb")
                    for m_ in range(2):
                        msl = slice(m_ * 128, (m_ + 1) * 128)
                        for k in range(4):
                            mm(fe1p[:, m_, :], fW1t[:, k, msl], ft[:, k, :], k == 0, k == 3)
                    fe1 = rot.tile([128, 2, 512], dt.bfloat16, tag="msg1", name="fe1")
                    for m_ in range(2):
                        act(fe1[:, m_, :], fe1p[:, m_, :], AF.Silu, bias=bt["fb1"][:, m_:m_ + 1])
                    fe2p = psA.tile([128, 2, 512], dt.float32, tag="accb", name="accb")
                    for m_ in range(2):
                        msl = slice(m_ * 128, (m_ + 1) * 128)
                        for k in range(2):
                            mm(fe2p[:, m_, :], fW2t[:, k, msl], fe1[:, k, :], k == 0, k == 1)
                    for m_ in range(2):
                        act(comb[:, 2 + m_, :], fe2p[:, m_, :], AF.Identity,
                            bias=bt["fb2"][:, m_:m_ + 1])

                    hp1 = big.tile([128, 6, 512], dt.bfloat16, tag="rz", name="hp1")
                    for mo in range(6):
                        hp1p = psA.tile([128, 512], dt.float32, tag="accb", name="accb")
                        for k in range(6):
                            mm(hp1p[:], pW1t[:, k, mo * 128:(mo + 1) * 128],
                               comb[:, k, :], k == 0, k == 5)
                        act(hp1[:, mo, :], hp1p[:], AF.Silu, bias=bt["pb1"][:, mo:mo + 1])
                    hp2p = psA.tile([128, 2, 512], dt.float32, tag="accb", name="accb")
                    for m_ in range(2):
                        msl = slice(m_ * 128, (m_ + 1) * 128)
                        for k in range(6):
                            mm(hp2p[:, m_, :], pW2t[:, k, msl], hp1[:, k, :], k == 0, k == 5)
                    hp2 = rot.tile([128, 2, 512], dt.bfloat16, tag="te", name="hp2")
                    for m_ in range(2):
                        act(hp2[:, m_, :], hp2p[:, m_, :], AF.Silu, bias=bt["pb2"][:, m_:m_ + 1])
                    h0p = psA.tile([128, 2, 512], dt.float32, tag="accb", name="accb")
                    for m_ in range(2):
                        msl = slice(m_ * 128, (m_ + 1) * 128)
                        for k in range(2):
                            mm(h0p[:, m_, :], pW3t[:, k, msl], hp2[:, k, :], k == 0, k == 1)
                    for m_ in range(2):
                        act(hT[:, m_, bsl], h0p[:, m_, :], AF.Identity,
                            bias=bt["pb3"][:, m_:m_ + 1])

            if "s0h" in dbg_out:
                nc.sync.dma_start(
                    dbg_out["s0h"][:].rearrange("p (a n) -> p a n", a=2), hT[:])
                nc.sync.dma_start(
                    dbg_out["s0x"][:].rearrange("p (g m) -> p g m", m=4), xb[:])

            # ============ cfg-resident edge tiles ============
            edgep = ctx.enter_context(tc.tile_pool(name="edgep", bufs=1))

            def load_idx(cfg):
                ne = NCH[cfg] * 128
                t = {}
                t["sel"] = edgep.tile([128, NCH[cfg], 256], dt.bfloat16, tag="sel", name="sel")
                nc.sync.dma_start(t["sel"][:], sel_d[cfg][:].rearrange("p (k j) -> p k j", j=256))
                t["rowi"] = edgep.tile([128, ne // 16], dt.int16, tag="rowi", name="rowi")
                nc.sync.dma_start(t["rowi"][:], rowi_d[cfg][:])
                t["coli"] = edgep.tile([128, ne // 16], dt.int16, tag="coli", name="coli")
                nc.sync.dma_start(t["coli"][:], coli_d[cfg][:])
                t["colx"] = edgep.tile([128, NCH[cfg]], dt.int32, tag="colx", name="colx")
                nc.sync.dma_start(t["colx"][:], colx_d[cfg][:])
                return t

            cfg_tiles = load_idx(0)
            cur_cfg = 0

            # ============ GCL layers ============

            for l in range(nl):
                cfg = 0 if (l // 3) % 2 == 0 else 1
                nch = NCH[cfg]
                ne = nch * 128
                nst = (nch + 3) // 4
                cpw = CPW[cfg]
                wstart = WSTART[cfg]
                if cfg != cur_cfg:
                    cfg_tiles = load_idx(cfg)
                    cur_cfg = cfg
                selt = cfg_tiles["sel"]
                # selT reloaded each layer (region shared with zrow)
                selTt = big.tile([128, nch, 256], dt.bfloat16, tag="rz", name="selTt")
                nc.sync.dma_start(selTt[:], selT_d[cfg][:].rearrange("p (k j) -> p k j", j=256))

                # --- layer weights ---
                We1t = wp.tile([128, 4, H], dt.bfloat16, tag="We1", name="We1")
                nc.sync.dma_start(We1t[:], We1t9[l][:].rearrange("(k p) m -> p k m", p=128))
                We1a = wp.tile([4, H], dt.bfloat16, tag="We1a", name="We1a")
                nc.sync.dma_start(We1a[:], We1a9[l][:])
                We2t = wp.tile([128, 2, H], dt.bfloat16, tag="We2", name="We2")
                nc.sync.dma_start(We2t[:], We29[l][:].rearrange("(k p) m -> p k m", p=128))
                Wn1t = wp.tile([128, 4, H], dt.bfloat16, tag="Wn1", name="Wn1")
                nc.sync.dma_start(Wn1t[:], Wn19[l][:].rearrange("(k p) m -> p k m", p=128))
                Wn2t = wp.tile([128, 2, H], dt.bfloat16, tag="Wn2", name="Wn2")
                nc.sync.dma_start(Wn2t[:], Wn29[l][:].rearrange("(k p) m -> p k m", p=128))
                Wc1t = wp.tile([128, 2, H], dt.bfloat16, tag="Wc1", name="Wc1")
                nc.sync.dma_start(Wc1t[:], Wc19[l][:].rearrange("(k p) m -> p k m", p=128))
                wattvt = wp.tile([128, 2], dt.bfloat16, tag="wattv", name="wattv")
                nc.sync.dma_start(wattvt[:], wattv9[l][:])
                wc2vt = wp.tile([128, 2], dt.bfloat16, tag="wc2v", name="wc2v")
                nc.sync.dma_start(wc2vt[:], wc2v9[l][:])
                be2t = wp.tile([128, 2], dt.float32, tag="be2", name="be2")
                nc.sync.dma_start(be2t[:], be29[l][:])
                bn1t = wp.tile([128, 2], dt.float32, tag="bn1", name="bn1")
                nc.sync.dma_start(bn1t[:], bn19[l][:])
                bc1t = wp.tile([128, 2], dt.float32, tag="bc1", name="bc1")
                nc.sync.dma_start(bc1t[:], bc19[l][:])
                bn2rt = wp.tile([1, H], dt.bfloat16, tag="bn2r", name="bn2r")
                nc.sync.dma_start(bn2rt[:], bn2r9[l][:])
                battt = wp.tile([1, 1], dt.float32, tag="batt", name="batt")
                nc.sync.dma_start(battt[:], batt9[l][:])

                # --- bounce: hTb -> node-major blocks -> DRAM; x hi/lo split ---
                for b in range(8):
                    hxb = rot.tile([128, 4, H], dt.bfloat16, tag="hxb", name="hxb")
                    for m_ in range(2):
                        tp = psA.tile([128, 512], dt.float32, tag="accb", name="accb")
                        for j in range(4):
                            g = b * 4 + j
                            nc.tensor.transpose(
                                out=tp[:, j * 128:(j + 1) * 128],
                                in_=hT[:, m_, g * 128:(g + 1) * 128],
                                identity=ident[:])
                        nc.any.tensor_copy(
                            hxb[:, :, m_ * 128:(m_ + 1) * 128],
                            tp[:].rearrange("p (j f) -> p j f", f=128))
                    nc.sync.dma_start(bounce_h[:, b * 4:(b + 1) * 4, :], hxb[:])
                nc.vector.tensor_copy(xbb2[:, :, 0:4], xb[:])
                xlo = rot.tile([128, G, 4], dt.float32, tag="xlo", name="xlo")
                nc.vector.tensor_tensor(out=xlo[:], in0=xb[:], in1=xbb2[:, :, 0:4],
                                        op=ALU.subtract)
                nc.vector.tensor_copy(xbb2[:, :, 4:8], xlo[:])
                nc.sync.dma_start(bounce_x[:], xbb2[:])

                if sim1:
                    hx_full = dram.tile([NC * 128, G, H], dt.bfloat16,
                                        tag="hxf", name="hxf")
                    nc.sync.dma_start(hx_full[0:128], bounce_h[:])
                    x_full = dram.tile([NC * 128, G, 8], dt.bfloat16,
                                       tag="xf", name="xf")
                    nc.sync.dma_start(x_full[0:128], bounce_x[:])
                else:
                    hx_full = dram.tile([NC * 128, G, H], dt.bfloat16,
                                        addr_space="Shared", tag=f"hxf{l}", name=f"hxf{l}")
                    x_full = dram.tile([NC * 128, G, 8], dt.bfloat16,
                                       addr_space="Shared", tag=f"xf{l}", name=f"xf{l}")
                    nc.gpsimd.collective_compute(
                        "AllGather", mybir.AluOpType.bypass,
                        replica_groups=[list(range(NC))],
                        ins=[bounce_x.opt()], outs=[x_full.opt()])
                    nc.gpsimd.collective_compute(
                        "AllGather", mybir.AluOpType.bypass,
                        replica_groups=[list(range(NC))],
                        ins=[bounce_h.opt()], outs=[hx_full.opt()])

                # --- P0: x gathers + batched radial chain (overlaps AG-h) ---
                x_rows = x_full[:].rearrange("p g m -> (p g) m")
                cx = big.tile([128, NCHMX, 8], dt.bfloat16, tag="cx", name="cx")
                xrs = big.tile([128, NCHMX, 8], dt.float32, tag="xrs", name="xrs")
                diff = big.tile([128, NCHMX, 4], dt.float32, tag="diff", name="diff")
                for k in range(nch):
                    nc.gpsimd.indirect_dma_start(
                        out=cx[:, k, :], out_offset=None, in_=x_rows,
                        in_offset=bass.IndirectOffsetOnAxis(
                            ap=cfg_tiles["colx"][:, k:k + 1], axis=0))
                    w = int(np.searchsorted(wstart, k, side="right") - 1)
                    xrp = psS.tile([128, 8], dt.float32, tag="s", name="xrp")
                    for hh in range(2):
                        mm(xrp[:], selTt[:, k, hh * 128:(hh + 1) * 128],
                           xbb2[:, 2 * w + hh, :], hh == 0, hh == 1)
                    nc.any.tensor_copy(xrs[:, k, :], xrp[:])
                cxf = big.tile([128, NCHMX, 8], dt.float32, tag="cxf", name="cxf")
                nc.vector.tensor_copy(cxf[:, 0:nch, :], cx[:, 0:nch, :])
                dAll = xrs
                nc.vector.tensor_tensor(out=dAll[:, 0:nch, :], in0=xrs[:, 0:nch, :],
                                        in1=cxf[:, 0:nch, :], op=ALU.subtract)
                nc.vector.tensor_tensor(out=diff[:, 0:nch, :], in0=dAll[:, 0:nch, 0:4],
                                        in1=dAll[:, 0:nch, 4:8], op=ALU.add)
                sq = big.tile([128, NCHMX, 4], dt.float32, tag="sq", name="sq")
                nc.vector.tensor_tensor(out=sq[:, 0:nch, :], in0=diff[:, 0:nch, :],
                                        in1=diff[:, 0:nch, :], op=ALU.mult)
                rad = big.tile([128, NCHMX], dt.float32, tag="rad", name="rad")
                nc.vector.tensor_reduce(out=rad[:, 0:nch], in_=sq[:, 0:nch, :],
                                        axis=mybir.AxisListType.X, op=ALU.add)
                den = big.tile([128, NCHMX], dt.float32, tag="den", name="den")
                act(den[:, 0:nch], rad[:, 0:nch], AF.Sqrt)
                nc.vector.tensor_scalar_add(out=den[:, 0:nch], in0=den[:, 0:nch],
                                            scalar1=1.0)
                rec = big.tile([128, NCHMX], dt.float32, tag="rec", name="rec")
                nc.vector.reciprocal(rec[:, 0:nch], den[:, 0:nch])
                cd = big.tile([128, NCHMX, 4], dt.float32, tag="cd", name="cd")
                nc.vector.tensor_tensor(out=cd[:, 0:nch, :], in0=diff[:, 0:nch, :],
                                        in1=rec[:, 0:nch].to_broadcast([128, nch, 4]),
                                        op=ALU.mult)
                nc.vector.tensor_copy(radx[:, 0:nch, 0:2],
                                      rad[:, 0:nch].to_broadcast([128, nch, 2]))

                # --- P1/P2: edge MLP + att raw (all-silu phase) ---
                msg2 = big.tile([128, 2, ne], dt.bfloat16, tag="msg2", name="msg2")
                attr = big.tile([1, ne], dt.bfloat16, tag="r1", name="attr")
                neb = (nch + EBC - 1) // EBC
                for eb in range(neb):
                    c0 = eb * EBC
                    ncb = min(EBC, nch - c0)
                    efT = big.tile([128, 4, ncb * 128], dt.bfloat16, tag="bigA",
                                   name="efT", bufs=2)
                    isl = slice(c0 * 8, c0 * 8 + ncb * 8)
                    nc.gpsimd.dma_gather(
                        out_ap=efT[:, 0:2, :],
                        in_ap=bounce_h[:].rearrange("p g m -> (p g) m"),
                        idxs_ap=cfg_tiles["rowi"][:, isl],
                        num_idxs=ncb * 128, num_idxs_reg=ncb * 128,
                        elem_size=H, transpose=True)
                    nc.gpsimd.dma_gather(
                        out_ap=efT[:, 2:4, :],
                        in_ap=hx_full[:].rearrange("p g m -> (p g) m"),
                        idxs_ap=cfg_tiles["coli"][:, isl],
                        num_idxs=ncb * 128, num_idxs_reg=ncb * 128,
                        elem_size=H, transpose=True)
                    for si in range((ncb + 3) // 4):
                        ch0 = c0 + si * 4
                        wch = min(4, nch - ch0) * 128
                        sl = slice(ch0 * 128, ch0 * 128 + wch)
                        esl = slice(si * 512, si * 512 + wch)
                        efr = rot.tile([4, 512], dt.bfloat16, tag="efr", name="efr")
                        for kj in range(wch // 128):
                            rp = psS.tile([4, 128], dt.bfloat16, tag="s", name="rT")
                            nc.tensor.transpose(out=rp[:], in_=radx[:, ch0 + kj, :],
                                                identity=identb[:])
                            nc.any.tensor_copy(efr[:, kj * 128:(kj + 1) * 128], rp[:])
                        m1p = psA.tile([128, 2, 512], dt.float32, tag="accb", name="accb")
                        for m_ in range(2):
                            msl = slice(m_ * 128, (m_ + 1) * 128)
                            for k in range(4):
                                mm(m1p[:, m_, :wch], We1t[:, k, msl], efT[:, k, esl],
                                   k == 0, False)
                            mm(m1p[:, m_, :wch], We1a[:, msl], efr[:, :wch], False, True)
                        msg1 = rot.tile([128, 2, 512], dt.bfloat16, tag="msg1", name="msg1")
                        act(msg1[:, :, :wch], m1p[:, :, :wch], AF.Silu)
                        m2p = psA.tile([128, 2, 512], dt.float32, tag="accb", name="accb")
                        for m_ in range(2):
                            msl = slice(m_ * 128, (m_ + 1) * 128)
                            for k in range(2):
                                mm(m2p[:, m_, :wch], We2t[:, k, msl], msg1[:, k, :wch],
                                   k == 0, k == 1)
                        for m_ in range(2):
                            act(msg2[:, m_, sl], m2p[:, m_, :wch], AF.Silu,
                                bias=be2t[:, m_:m_ + 1])
                        attp = psS.tile([1, 512], dt.float32, tag="s", name="attp")
                        for m_ in range(2):
                            mm(attp[:, :wch], wattvt[:, m_:m_ + 1], msg2[:, m_, sl],
                               m_ == 0, m_ == 1)
                        nc.any.tensor_copy(attr[:, sl], attp[:, :wch])

                # --- P3: sigmoid (one table load) ---
                atts = big.tile([1, ne], dt.bfloat16, tag="r2", name="atts")
                act(atts[:], attr[:], AF.Sigmoid, bias=battt[:, 0:1])

                # --- P4/P5: msge (in place) + coord weight path ---
                zrow = big.tile([1, ne], dt.bfloat16, tag="rz", name="zrow")
                for st in range(nst):
                    ch0 = st * 4
                    wch = min(4, nch - ch0) * 128
                    sl = slice(ch0 * 128, ch0 * 128 + wch)
                    abc = psS.tile([128, 512], dt.float32, tag="s", name="abc")
                    mm(abc[:, :wch], ones128t[:], atts[:, sl], True, True)
                    abcb = rot.tile([128, 512], dt.bfloat16, tag="abcb", name="abcb")
                    nc.any.tensor_copy(abcb[:, :wch], abc[:, :wch])
                    for m_ in range(2):
                        nc.vector.tensor_tensor(out=msg2[:, m_, sl], in0=msg2[:, m_, sl],
                                                in1=abcb[:, :wch], op=ALU.mult)
                    wep = psA.tile([128, 2, 512], dt.float32, tag="accb", name="accb")
                    for m_ in range(2):
                        msl = slice(m_ * 128, (m_ + 1) * 128)
                        for k in range(2):
                            mm(wep[:, m_, :wch], Wc1t[:, k, msl], msg2[:, k, sl],
                               k == 0, k == 1)
                    te = rot.tile([128, 2, 512], dt.bfloat16, tag="te", name="te")
                    for m_ in range(2):
                        act(te[:, m_, :wch], wep[:, m_, :wch], AF.Silu,
                            bias=bc1t[:, m_:m_ + 1])
                    zp = psS.tile([1, 512], dt.float32, tag="s", name="zp")
                    for m_ in range(2):
                        mm(zp[:, :wch], wc2vt[:, m_:m_ + 1], te[:, m_, :wch],
                           m_ == 0, m_ == 1)
                    nc.any.tensor_copy(zrow[:, sl], zp[:, :wch])
                msge = msg2

                # --- P6: tanh (one table load), scale by COORDS_RANGE ---
                th = big.tile([1, ne], dt.bfloat16, tag="r1", name="th")
                act(th[:], zrow[:], AF.Tanh)
                nc.vector.tensor_scalar_mul(out=th[:], in0=th[:],
                                            scalar1=float(COORDS_RANGE))

                # --- P7a: x scatter (PSUM chains per group) ---
                xps = psA.tile([128, G, 4], dt.float32, tag="accb", name="accb")
                cdt = big.tile([128, NCHMX, 4], dt.bfloat16, tag="cdt", name="cdt")
                for k in range(nch):
                    thT = psS.tile([128, 1], dt.bfloat16, tag="s", name="thT")
                    nc.tensor.transpose(out=thT[:], in_=th[:, k * 128:(k + 1) * 128],
                                        identity=identb[:1, :1])
                    thf = rot.tile([128, 1], dt.float32, tag="thf", name="thf")
                    nc.any.tensor_copy(thf[:], thT[:])
                    nc.vector.tensor_tensor(out=cdt[:, k, :], in0=cd[:, k, :],
                                            in1=thf[:, 0:1].to_broadcast([128, 4]),
                                            op=ALU.mult)
                for w in range(NW):
                    ks = list(range(wstart[w], wstart[w] + cpw[w]))
                    for hh in range(2):
                        g = 2 * w + hh
                        for ki, k in enumerate(ks):
                            mm(xps[:, g, :], selt[:, k, hh * 128:(hh + 1) * 128],
                               cdt[:, k, :], ki == 0, ki == len(ks) - 1)
                nc.vector.tensor_tensor(out=xb[:], in0=xb[:], in1=xps[:], op=ALU.add)

                # --- P7b/P8: h scatter chains + node MLP per 512-node block ---
                for b in range(8):
                    bsl = slice(b * 512, (b + 1) * 512)
                    agg = psG.tile([128, 2, 512], dt.float32, tag="agg", name="agg")
                    for wo in range(2):
                        w = 2 * b + wo
                        ks = list(range(wstart[w], wstart[w] + cpw[w]))
                        for ki, k in enumerate(ks):
                            mep = psS.tile([128, 256], dt.bfloat16, tag="s", name="mep")
                            for m_ in range(2):
                                nc.tensor.transpose(
                                    out=mep[:, m_ * 128:(m_ + 1) * 128],
                                    in_=msge[:, m_, k * 128:(k + 1) * 128],
                                    identity=identb[:])
                            me = rot.tile([128, 256], dt.bfloat16, tag="me", name="me",
                                          bufs=3)
                            nc.any.tensor_copy(me[:], mep[:])
                            for m_ in range(2):
                                mm(agg[:, m_, wo * 256:(wo + 1) * 256],
                                   me[:, m_ * 128:(m_ + 1) * 128], selt[:, k, :],
                                   ki == 0, ki == len(ks) - 1)
                    aggsb = rot.tile([128, 2, 512], dt.bfloat16, tag="aggsb", name="aggsb")
                    nc.any.tensor_copy(aggsb[:], agg[:])
                    hTbb = rot.tile([128, 2, 512], dt.bfloat16, tag="hTbb", name="hTbb")
                    nc.any.tensor_copy(hTbb[:], hT[:, :, bsl])
                    n1p = psA.tile([128, 2, 512], dt.float32, tag="accb", name="accb")
                    for m_ in range(2):
                        msl = slice(m_ * 128, (m_ + 1) * 128)
                        for k in range(2):
                            mm(n1p[:, m_, :], Wn1t[:, k, msl], hTbb[:, k, :], k == 0, False)
                        for k in range(2):
                            mm(n1p[:, m_, :], Wn1t[:, 2 + k, msl], aggsb[:, k, :],
                               False, k == 1)
                    nh1 = rot.tile([128, 2, 512], dt.bfloat16, tag="nh1", name="nh1")
                    for m_ in range(2):
                        act(nh1[:, m_, :], n1p[:, m_, :], AF.Silu, bias=bn1t[:, m_:m_ + 1])
                    n2p = psA.tile([128, 2, 512], dt.float32, tag="accb", name="accb")
                    for m_ in range(2):
                        msl = slice(m_ * 128, (m_ + 1) * 128)
                        for k in range(2):
                            mm(n2p[:, m_, :], Wn2t[:, k, msl], nh1[:, k, :], k == 0, False)
                        mm(n2p[:, m_, :], bn2rt[:, msl], ones512t[:], False, True)
                    nc.vector.tensor_tensor(out=hT[:, :, bsl], in0=hT[:, :, bsl],
                                            in1=n2p[:], op=ALU.add)

                nm = f"s{l + 1}"
                if nm + "h" in dbg_out:
                    nc.sync.dma_start(
                        dbg_out[nm + "h"][:].rearrange("p (a n) -> p a n", a=2), hT[:])
                    nc.sync.dma_start(
                        dbg_out[nm + "x"][:].rearrange("p (g m) -> p g m", m=4), xb[:])

            # ============ output head ============
            if with_head:
                for b in range(8):
                    hxb = rot.tile([128, 4, H], dt.bfloat16, tag="hxb", name="hxb")
                    for m_ in range(2):
                        tp = psA.tile([128, 512], dt.float32, tag="accb", name="accb")
                        for j in range(4):
                            g = b * 4 + j
                            nc.tensor.transpose(
                                out=tp[:, j * 128:(j + 1) * 128],
                                in_=hT[:, m_, g * 128:(g + 1) * 128],
                                identity=ident[:])
                        nc.any.tensor_copy(
                            hxb[:, :, m_ * 128:(m_ + 1) * 128],
                            tp[:].rearrange("p (j f) -> p j f", f=128))
                    nc.sync.dma_start(bounce_h[:, b * 4:(b + 1) * 4, :], hxb[:])

                oW1t = wp.tile([128, 2, H], dt.bfloat16, tag="We2", name="oW1t")
                nc.sync.dma_start(oW1t[:], oW1[:].rearrange("(k p) m -> p k m", p=128))
                oW1vt = wp.tile([1, H], dt.bfloat16, tag="oW1v", name="oW1v")
                nc.sync.dma_start(oW1vt[:], oW1v[:])
                oW2t = wp.tile([128, 2, VOCAB], dt.bfloat16, tag="oW2", name="oW2t")
                nc.sync.dma_start(oW2t[:], oW2[:].rearrange("(k p) m -> p k m", p=128))
                ob1t = wp.tile([128, 2], dt.float32, tag="be2", name="ob1t")
                nc.sync.dma_start(ob1t[:], ob1[:])
                ob2t = wp.tile([128, 7], dt.float32, tag="ob2", name="ob2t")
                nc.sync.dma_start(ob2t[:], ob2[:])
                pidxt = rot.tile([4, 1], dt.int32, tag="pidxt", name="pidxt")
                nc.sync.dma_start(pidxt[:], pidx[:])
                valt = rot.tile([1, 4], dt.float32, tag="valt", name="valt")
                nc.sync.dma_start(valt[:], valrow[:])
                valb = rot.tile([1, 4], dt.bfloat16, tag="valb", name="valb")
                nc.vector.tensor_copy(valb[:], valt[:])

                hsel = rot.tile([4, H], dt.bfloat16, tag="hsel", name="hsel")
                nc.gpsimd.indirect_dma_start(
                    out=hsel[:], out_offset=None,
                    in_=bounce_h[:].rearrange("p g m -> (p g) m"),
                    in_offset=bass.IndirectOffsetOnAxis(ap=pidxt[:, :1], axis=0))
                hselT = rot.tile([128, 2, 4], dt.bfloat16, tag="hselT", name="hselT")
                for m_ in range(2):
                    tp = psS.tile([128, 4], dt.bfloat16, tag="s", name="tp4")
                    nc.tensor.transpose(out=tp[:], in_=hsel[:, m_ * 128:(m_ + 1) * 128],
                                        identity=identb[:4, :4])
                    nc.any.tensor_copy(hselT[:, m_, :], tp[:])
                o1p = psS.tile([128, 2, 4], dt.float32, tag="s", name="o1p")
                for m_ in range(2):
                    msl = slice(m_ * 128, (m_ + 1) * 128)
                    for k in range(2):
                        mm(o1p[:, m_, :], oW1t[:, k, msl], hselT[:, k, :], k == 0, False)
                    mm(o1p[:, m_, :], oW1vt[:, msl], valb[:], False, True)
                o1 = rot.tile([128, 2, 4], dt.bfloat16, tag="o1", name="o1")
                for m_ in range(2):
                    act(o1[:, m_, :], o1p[:, m_, :], AF.Silu, bias=ob1t[:, m_:m_ + 1])
                hout = rot.tile([4, VOCAB], dt.float32, tag="hout", name="hout")
                for mo in range(7):
                    mw = min(128, VOCAB - mo * 128)
                    o2p = psS.tile([128, 4], dt.float32, tag="s", name="o2p")
                    for k in range(2):
                        mm(o2p[:mw, :], oW2t[:, k, mo * 128:mo * 128 + mw], o1[:, k, :],
                           k == 0, k == 1)
                    o2b = rot.tile([128, 4], dt.float32, tag="o2b", name="o2b")
                    act(o2b[:mw, :], o2p[:mw, :], AF.Identity, bias=ob2t[:mw, mo:mo + 1])
                    fp = psS.tile([4, 128], dt.float32, tag="s", name="fp")
                    nc.tensor.transpose(out=fp[:, :mw], in_=o2b[:mw, :],
                                        identity=ident[:mw, :mw])
                    nc.any.tensor_copy(hout[:, mo * 128:mo * 128 + mw], fp[:, :mw])
                nc.sync.dma_start(head_out[:], hout[:])
            else:
                zt = rot.tile([4, VOCAB], dt.float32, tag="zt", name="zt")
                nc.gpsimd.memset(zt[:], 0.0)
                nc.sync.dma_start(head_out[:], zt[:])

    nc.compile()
    return nc


def build_and_run(inputs, nl=N_LAYERS, with_head=True, dbg=(), trace=False):
    from concourse.bass_utils import run_bass_kernel_spmd
    meta, maps = _prep(inputs)
    key = (nl, with_head, tuple(dbg))
    if key not in _cache:
        _cache[key] = _build(meta, nl=nl, with_head=with_head, dbg=dbg)
    nc = _cache[key]
    res = run_bass_kernel_spmd(nc, maps, core_ids=list(range(NC)), trace=trace)
    return res


def decode_h(arr):
    """dbg [128, 2*NS] feature-major -> h [NS, 256]"""
    a = arr.reshape(128, 2, NS)
    return np.concatenate([a[:, 0, :].T, a[:, 1, :].T], axis=1)


def decode_x(arr):
    """dbg [128, G*4] node-major -> x [NS, 3]"""
    a = arr.reshape(128, G, 4).transpose(1, 0, 2).reshape(NS, 4)
    return a[:, :3]


def kernel(**inputs) -> np.ndarray:
    res = build_and_run(inputs)
    out = np.concatenate([res.results[c]["head_out"] for c in range(NC)], 0)
    return out.astype(np.float32)
